# revision 28
# baseline (speedup 1.0000x reference)
"""Trainium2 Bass kernel for nn_PrototypicalGeometricLoss (v3).

Strategy (8 NeuronCores, single NEFF launch):

  - Data-parallel streaming: each core mean-pools + L2-normalizes its
    B/8 = 512 rows of geometric_stream (bf16 in-flight DMA cast, PE pooling
    against a constant selection matrix) and computes distances to the
    never-updated classes (bulk) during the stream; phase A + bulk hide
    completely under the 41us DMA floor.
  - ONE post-stream AllGather of the normalized embeddings in fp8e4m3
    ([b,d] orientation, 512KB wire); collectives fired mid-stream contend
    with the saturated HBM path, and per-collective fixed cost dominates,
    so a single late gather beats four pipelined ones.  The assemble DMA
    (gpsimd SWDGE) casts fp8 -> bf16 in flight.
  - Segment sums for the EMA update are PE matmuls against a host-built
    one-hot selection matrix SEL [4096, NUPAD] (bf16) with the per-class
    EMA scale (1-m)/(m*count) folded into its values; old prototypes are
    injected into the same PSUM accumulation by an identity-stationary
    matmul, so ptil = p_old + scaled sums appears directly in PSUM (no
    gather/scan chain, no separate EMA add).  [d,b]-oriented tiles for the
    distance matmuls come from 32 PE transposes of the gathered tiles
    (dma_start_transpose races the collective - untracked dependency).
  - Renormalization avoids a broadcast matrix: phase F runs in swapped
    orientation (classes on partitions), so 2/||ptil_c|| is a per-partition
    activation `scale` vector (column-sum matmuls + Sqrt(0.25 x) + DVE
    reciprocal), and pad-class rows are excluded exactly by per-block host
    slicing.
  - Own-class dots: member columns of gTw (f32 copy of gathered gT)
    gathered once on gpsimd, multiplied elementwise with gathered -ptil
    columns, column-summed by a ones matmul; raw dots + per-class
    2/||ptil|| return to the host, which finishes in float64.
  - All post-collective work is emitted after the stream loop in
    dependency-monotone order: engine queues are in-order, and ~2.5us
    cross-engine latency per dependent instruction makes a short, batched
    tail the dominant design constraint.
"""

import functools
import sys

sys.path.insert(0, "/opt/trn_rl_repo")

import numpy as np
import ml_dtypes

import concourse.bass as bass  # noqa: F401
import concourse.bacc as bacc
import concourse.mybir as mybir
from concourse import tile
from concourse.bass_utils import run_bass_kernel_spmd

N_CORES = 8
B, S, D, C = 4096, 64, 128, 10000
BSH = B // N_CORES           # 512 batch rows per core
LTB = BSH // 128             # 4 local b-tiles
NTB = B // 128               # 32 global b-tiles
CSH = C // N_CORES           # 1250 classes per core
GCOL = NTB * 128             # 4096 columns of gT
MOM = 0.9
GSCALE = 1.0 - 1e-6          # keeps 2 - 2*g.p strictly positive
QR = 2048                    # gs quarter-row length (16 s x 128 d)
CHW = 1024                   # gT columns per AllGather chunk

f32 = mybir.dt.float32
fp8 = mybir.dt.float8e4
bf16 = mybir.dt.bfloat16
i16 = mybir.dt.int16
AF = mybir.ActivationFunctionType
ALU = mybir.AluOpType
AX = mybir.AxisListType


def _wrap16(flat):
    """Lay a flat index list out in the GPSIMD wrapped-by-16 format."""
    n = flat.shape[0]
    assert n % 16 == 0
    w = flat.reshape(n // 16, 16).T.astype(np.int16)   # [16, n//16]
    return np.tile(w, (8, 1))                          # [128, n//16]


@functools.lru_cache(maxsize=16)
def _build(L, NUPAD, NBK, NBREAL, use_f32r=True, upto=99, unroll=1,
           skip_ag=False, dma_only=False, dma2x=False):
    """Build + compile the SPMD program.

    L = padded member count per AllGather chunk; NUPAD = padded
    updated-classes-per-shard block; NBK/NBREAL = padded/real count of
    globally-never-updated classes (the b-sharded bulk pass)."""
    NOWN = 4 * L
    NCB = NUPAD // 128                    # class blocks (partition tiles)
    # po packs per-(class-block, group) F sums at cols 3..3+NCB*NFG and
    # rinv2 at 16..16+NCB; bigger NUPAD must fail loudly here
    assert NCB <= 4, "po column layout requires NCB <= 4"
    bulk_chunks = []                      # (c0, width) into prTg
    for c0 in range(0, NBK, 1536):
        bulk_chunks.append((c0, min(1536, NBK - c0)))
    NBG = LTB * len(bulk_chunks)
    f_groups = []                         # (k0, width) into gT columns
    for k0 in range(0, GCOL, 1536):
        f_groups.append((k0, min(1536, GCOL - k0)))
    NFG = len(f_groups)                   # 3 groups of <=1536 cols
    NAC = NBG

    nc = bacc.Bacc("TRN2", target_bir_lowering=False, debug=False,
                   num_devices=N_CORES)

    gs = nc.dram_tensor("gs", [BSH * 4, QR], f32, kind="ExternalInput")
    sel = nc.dram_tensor("sel", [128, NTB * NUPAD], bf16,
                         kind="ExternalInput")
    prTb = nc.dram_tensor("prTb", [128, NUPAD], bf16, kind="ExternalInput")
    prTg = nc.dram_tensor("prTg", [128, NBK], bf16, kind="ExternalInput")
    sv = nc.dram_tensor("sv", [128, BSH // 128], f32, kind="ExternalInput")
    mgi = nc.dram_tensor("mgi", [128, NOWN // 16], i16, kind="ExternalInput")
    opi = nc.dram_tensor("opi", [128, NOWN // 16], i16, kind="ExternalInput")
    idn = nc.dram_tensor("idn", [128, 128], f32, kind="ExternalInput")
    idnb = nc.dram_tensor("idnb", [128, 128], bf16, kind="ExternalInput")
    osel = nc.dram_tensor("osel", [128, 32], bf16, kind="ExternalInput")
    po = nc.dram_tensor("po", [128, 20], f32, kind="ExternalOutput")
    oo = nc.dram_tensor("oo", [1, NOWN], f32, kind="ExternalOutput")

    with tile.TileContext(nc) as tc:
        with (
            tc.tile_pool(name="dram", bufs=1, space="DRAM") as dram,
            tc.tile_pool(name="consts", bufs=1) as consts,
            tc.tile_pool(name="gbig", bufs=1) as gbig,
            tc.tile_pool(name="slab", bufs=8) as slabp,
            tc.tile_pool(name="slabf", bufs=2) as slabfp,
            tc.tile_pool(name="selp", bufs=2) as selp,
            tc.tile_pool(name="norm", bufs=2) as normp,
            tc.tile_pool(name="glocp", bufs=4) as glocp,
            tc.tile_pool(name="ps_small", bufs=1, space="PSUM") as ps_small,
            tc.tile_pool(name="ps_big", bufs=2, space="PSUM") as ps_big,
            tc.tile_pool(name="ps_seg", bufs=1, space="PSUM") as ps_seg,
            tc.tile_pool(name="dscr", bufs=2) as dscrp,
            tc.tile_pool(name="outs", bufs=1) as outsp,
        ):
            for it in range(unroll):
                if it > 0:
                    tc.strict_bb_all_engine_barrier()
                ag_in = dram.tile([128, LTB * 128], fp8, name="ag_in",
                                  tag="ag_in")
                ag_out = dram.tile([N_CORES, 128, LTB * 128], fp8,
                                   name="ag_out", addr_space="Shared",
                                   tag="ag_out")
                ident = consts.tile([128, 128], f32, name="ident")
                nc.sync.dma_start(ident[:, :], idn[:, :])
                identb = consts.tile([128, 128], bf16, name="identb")
                nc.sync.dma_start(identb[:, :], idnb[:, :])
                osel_sb = consts.tile([128, 32], bf16, name="osel_sb")
                nc.sync.dma_start(osel_sb[:, :], osel[:, :])
                mgi_sb = consts.tile([128, NOWN // 16], i16, name="mgi_sb")
                nc.sync.dma_start(mgi_sb[:, :], mgi[:, :])
                opi_sb = consts.tile([128, NOWN // 16], i16, name="opi_sb")
                nc.sync.dma_start(opi_sb[:, :], opi[:, :])
                sv_sb = consts.tile([128, BSH // 128], f32, name="sv_sb")
                nc.sync.dma_start(sv_sb[:, :], sv[:, :])
                bias2 = consts.tile([128, 1], f32, name="bias2")
                nc.vector.memset(bias2[:, :], 2.0)
                onescol = consts.tile([128, 1], f32, name="onescol")
                nc.vector.memset(onescol[:, :], 1.0)
                prT_sb = consts.tile([128, NUPAD], bf16, name="prT_sb")
                nc.sync.dma_start(prT_sb[:, :], prTb[:, :])
                prTg_sb = consts.tile([128, NBK], bf16, name="prTg_sb")
                nc.scalar.dma_start(prTg_sb[:, :], prTg[:, :])

                gT = gbig.tile([128, GCOL], bf16, name="gT")
                gTw = gbig.tile([128, GCOL], f32, name="gTw")
                gnAll = gbig.tile([128, GCOL], bf16, name="gnAll")
                mems = gbig.tile([128, NOWN], f32, name="mems")
                acc = outsp.tile([128, NAC], f32, name="acc")
                po_sb = outsp.tile([128, 20], f32, name="po_sb")
                oo_sb = outsp.tile([1, NOWN], f32, name="oo_sb")
                if upto < 1:
                    nc.vector.memset(acc[:, 0:NBG], 0.0)

                seg_ps = ps_seg.tile([128, NUPAD], f32, name="seg_ps")
                if upto >= 3:
                    # inject p_old into the EMA accumulation group
                    nc.tensor.matmul(seg_ps[:, :], identb[:, :], prT_sb[:, :],
                                     start=True, stop=False,
                                     skip_group_check=True)


                def transp(t):
                    for j in range(N_CORES):
                        blk = t * N_CORES + j
                        pst2 = ps_big.tile([128, 128], bf16, name="pst2",
                                           tag="psb")
                        nc.tensor.transpose(pst2[:, :],
                                            gnAll[:, blk * 128:
                                                  (blk + 1) * 128],
                                            identb[:, :])
                        nc.vector.tensor_scalar_mul(
                            gT[:, blk * 128:(blk + 1) * 128], pst2[:, :], 1.0)

                def memgather():
                    nc.gpsimd.ap_gather(mems[:, :], gTw[:, :],
                                        mgi_sb[:, :],
                                        channels=128, num_elems=GCOL, d=1,
                                        num_idxs=NOWN)

                def segsum(t, sel_sb):
                    for blk in range(N_CORES):
                        g0 = (t * N_CORES + blk) * 128
                        nc.tensor.matmul(
                            seg_ps[:, :],
                            gnAll[:, g0:g0 + 128],
                            sel_sb[:, blk * NUPAD:(blk + 1) * NUPAD],
                            start=False,
                            stop=(t == LTB - 1 and blk == N_CORES - 1),
                            skip_group_check=True)

                # ---- Phase A: stream + PE-pool + normalize + AG + bulk
                sel_tiles = []
                glocs = []
                for t in range(LTB if (upto >= 1 or dma_only) else 0):
                    if upto >= 3:
                        sel_sb = selp.tile([128, N_CORES * NUPAD], bf16,
                                           name="sel_sb", tag="sel")
                        nc.sync.dma_start(
                            sel_sb[:, :],
                            sel[:, t * N_CORES * NUPAD:
                                (t + 1) * N_CORES * NUPAD])
                        sel_tiles.append(sel_sb)
                    pooled = ps_small.tile([128, 128], f32, name="pooled",
                                           tag="pst")
                    hs = []
                    for h in range(2):
                        sh = slabp.tile([128, 2 * QR], bf16, name="sh",
                                        tag="slab")
                        r0 = t * 512 + h * 256
                        nc.gpsimd.dma_start(
                            sh[:, :].rearrange("p (tl f) -> p tl f", tl=2),
                            gs[r0:r0 + 256, :].rearrange(
                                "(tl p) f -> p tl f", tl=2))
                        if dma2x:
                            # probe: duplicate read on the SP HWDGE queue
                            # (f32, no cast) to test dual-queue bandwidth
                            sf = slabfp.tile([128, 2 * QR], f32, name="sf",
                                             tag="slabf")
                            nc.sync.dma_start(
                                sf[:, :].rearrange("p (tl f) -> p tl f",
                                                   tl=2),
                                gs[r0:r0 + 256, :].rearrange(
                                    "(tl p) f -> p tl f", tl=2))
                        hs.append(sh)
                    if dma_only:
                        continue
                    for tl in range(4):
                        sh = hs[tl // 2]
                        o = (tl % 2) * QR
                        for q in range(16):
                            nc.tensor.matmul(
                                pooled[tl * 32:(tl + 1) * 32, :],
                                osel_sb[:, :],
                                sh[:, o + q * 128:o + (q + 1) * 128],
                                start=(q == 0), stop=(q == 15),
                                tile_position=(0, tl * 32))
                    scr = normp.tile([128, 128], f32, name="scr", tag="scr")
                    ssq = normp.tile([128, 1], f32, name="ssq", tag="ssq")
                    nc.scalar.activation(scr[:, :], pooled[:, :], AF.Square,
                                         accum_out=ssq[:, :])
                    nrm = normp.tile([128, 1], f32, name="nrm", tag="nrm")
                    nc.scalar.activation(nrm[:, :], ssq[:, :], AF.Sqrt)
                    rcp = normp.tile([128, 1], f32, name="rcp", tag="rcp")
                    nc.vector.reciprocal(rcp[:, :], nrm[:, :])
                    gnb = normp.tile([128, 128], fp8, name="gnb", tag="gnb")
                    nc.vector.tensor_scalar(gnb[:, :], pooled[:, :],
                                            rcp[:, :], GSCALE, ALU.mult,
                                            ALU.mult)
                    gn = normp.tile([128, 128], f32, name="gn", tag="gn")
                    nc.vector.tensor_scalar(gn[:, :], pooled[:, :], rcp[:, :],
                                            GSCALE, ALU.mult, ALU.mult)
                    pst = ps_small.tile([128, 128], f32, name="pst", tag="pst")
                    nc.tensor.transpose(pst[:, :], gn[:, :], ident[:, :])
                    gloc = glocp.tile([128, 128], bf16, name="gloc",
                                      tag="gloc")
                    nc.scalar.activation(gloc[:, :], pst[:, :], AF.Copy)
                    nc.sync.dma_start(ag_in[:, t * 128:(t + 1) * 128],
                                      gnb[:, :])
                    glocs.append(gloc)

                    # bulk: local b-tile x never-updated classes
                    for ci, (c0, cw) in enumerate(bulk_chunks):
                        psb = ps_big.tile([128, 1536], f32, name="psb",
                                          tag="psb")
                        for d0 in range(0, cw, 512):
                            dn = min(512, cw - d0)
                            nc.tensor.matmul(
                                psb[:, d0:d0 + dn],
                                gloc[:, :],
                                prTg_sb[:, c0 + d0:c0 + d0 + dn],
                                start=True, stop=True)
                        rw = min(cw, max(0, NBREAL - c0))
                        dscb = dscrp.tile([128, 1536], bf16, name="dscb",
                                          tag="dsc")
                        gidx = t * len(bulk_chunks) + ci
                        nc.scalar.activation(dscb[:, 0:rw], psb[:, 0:rw],
                                             AF.Sqrt, bias=bias2[:, :],
                                             scale=-2.0,
                                             accum_out=acc[:, gidx:gidx + 1])

                # post-stream: assemble chunks, then chunk segsums in
                # dependency order (in-order engine queues must not stall
                # behind collective waits during the stream; deps are
                # monotone in t, so queues drain without head blocking)
                if upto >= 2 and not dma_only:
                    # one collective fired post-stream: pays the per-AG fixed
                    # cost once; everything downstream waits on the last tile
                    # anyway and is latency-bound
                    if not skip_ag:
                        nc.gpsimd.collective_compute(
                            "AllGather", ALU.bypass,
                            replica_groups=[list(range(N_CORES))],
                            ins=[ag_in.opt()], outs=[ag_out.opt()])
                    nc.gpsimd.dma_start(
                        gnAll[:, :].rearrange("p (t j c) -> p t j c",
                                              t=LTB, j=N_CORES),
                        ag_out[:, :, :].rearrange("j p (t c) -> p t j c",
                                                  t=LTB))
                    if upto >= 3:
                        for t in range(LTB):
                            segsum(t, sel_tiles[t])

                # bulk reduce: input complete at stream end; emitted
                # here so it executes during the AllGather wait
                nc.vector.tensor_reduce(po_sb[:, 0:1], acc[:, 0:NBG], AX.X,
                                        ALU.add)

                # ---- ptil -> norms -> rinv2 (per-class 2/||ptil||)
                ptilb = gbig.tile([128, NUPAD], bf16, name="ptilb")
                ptn = gbig.tile([128, NUPAD], f32, name="ptn")
                sqt = gbig.tile([128, NUPAD], f32, name="sqt")
                srow = outsp.tile([128, NCB], f32, name="srow")
                if upto >= 3:
                    nc.vector.tensor_scalar_mul(ptilb[:, :], seg_ps[:, :],
                                                -1.0)
                    nc.vector.tensor_scalar_mul(ptn[:, :], seg_ps[:, :], -1.0)
                    nc.scalar.activation(sqt[:, :], ptilb[:, :], AF.Square)
                    nps = ps_small.tile([128, NCB], f32, name="nps",
                                        tag="pst")
                    for cb in range(NCB):
                        nc.tensor.matmul(nps[:, cb:cb + 1],
                                         sqt[:, cb * 128:(cb + 1) * 128],
                                         onescol[:, :],
                                         start=True, stop=True)
                    nc.scalar.activation(srow[:, :], nps[:, :], AF.Sqrt,
                                         scale=0.25)
                    nc.vector.reciprocal(po_sb[:, 16:16 + NCB], srow[:, :])
                else:
                    nc.vector.memset(ptilb[:, :], 0.0)
                    nc.vector.memset(ptn[:, :], 0.0)
                    nc.vector.memset(po_sb[:, 16:16 + NCB], 0.0)

                if upto >= 2 and not dma_only:
                    for t in range(LTB):
                        transp(t)
                    nc.vector.tensor_scalar_mul(gTw[:, :], gT[:, :], 1.0)
                    memgather()

                # ---- simplex volume partials
                nc.vector.tensor_reduce(po_sb[:, 1:2], sv_sb[:, :], AX.X,
                                        ALU.add)
                junk1 = outsp.tile([128, BSH // 128], f32, name="junk1")
                nc.vector.scalar_tensor_tensor(junk1[:, :], sv_sb[:, :], 1.0,
                                               sv_sb[:, :], ALU.mult,
                                               ALU.mult,
                                               accum_out=po_sb[:, 2:3])
                nc.vector.memset(po_sb[:, 3 + NCB * NFG:16], 0.0)

                # ---- Phase F: distances for updated classes (swapped)
                if upto < 4:
                    nc.vector.memset(po_sb[:, 3:3 + NCB * NFG], 0.0)
                for cb in range(NCB if upto >= 4 else 0):
                    for gi, (k0, w) in enumerate(f_groups):
                        psF = ps_big.tile([128, 1536], f32, name="psF",
                                          tag="psb")
                        for d0 in range(0, w, 512):
                            dn = min(512, w - d0)
                            nc.tensor.matmul(
                                psF[:, d0:d0 + dn],
                                ptilb[:, cb * 128:(cb + 1) * 128],
                                gT[:, k0 + d0:k0 + d0 + dn],
                                start=True, stop=True)
                        dscf = dscrp.tile([128, 1536], bf16, name="dscf",
                                          tag="dsc")
                        aidx = 3 + cb * NFG + gi
                        nc.scalar.activation(dscf[:, 0:w], psF[:, 0:w],
                                             AF.Sqrt, bias=bias2[:, :],
                                             scale=po_sb[:, 16 + cb:17 + cb],
                                             accum_out=po_sb[:, aidx:aidx + 1])

                # ---- own-class raw dots: mems . gathered(-ptil)
                opg = gbig.tile([128, NOWN], f32, name="opg")
                prod = gbig.tile([128, NOWN], f32, name="prod")
                if upto >= 5:
                    nc.gpsimd.ap_gather(opg[:, :], ptn[:, :], opi_sb[:, :],
                                        channels=128, num_elems=NUPAD, d=1,
                                        num_idxs=NOWN)
                    nc.vector.tensor_tensor(prod[:, :], mems[:, :],
                                            opg[:, :], ALU.mult)
                    for o0 in range(0, NOWN, 512):
                        on = min(512, NOWN - o0)
                        dps = ps_big.tile([1, 512], f32, name="dps",
                                          tag="psb")
                        nc.tensor.matmul(dps[0:1, 0:on], onescol[:, :],
                                         prod[:, o0:o0 + on],
                                         start=True, stop=True)
                        nc.vector.tensor_scalar_mul(oo_sb[0:1, o0:o0 + on],
                                                    dps[0:1, 0:on], 1.0)
                else:
                    nc.vector.memset(oo_sb[:, :], 0.0)
                nc.sync.dma_start(oo[:, :], oo_sb[:, :])

                nc.sync.dma_start(po[:, :], po_sb[:, :])

    nc.compile()
    return nc


def _prep(geometric_stream, simplex_volumes, prototypes, labels):
    gs = np.ascontiguousarray(np.asarray(geometric_stream, dtype=np.float32))
    svol = np.ascontiguousarray(np.asarray(simplex_volumes, dtype=np.float32))
    pr = np.asarray(prototypes, dtype=np.float32)
    lab = np.asarray(labels).astype(np.int64).ravel()
    assert gs.shape == (B, S, D) and pr.shape == (C, D) and lab.shape == (B,)

    counts = np.bincount(lab, minlength=C)
    sscale = ((1.0 - MOM) / np.maximum(counts, 1.0) / MOM).astype(np.float64)

    shard_of = lab // CSH
    # osel[p, m] = 1 iff p//4 == m  (sums the 4 s-quarters of batch m)
    osel = (np.arange(128)[:, None] // 4 == np.arange(32)[None, :])
    osel = osel.astype(ml_dtypes.bfloat16)
    ident = np.eye(128, dtype=np.float32)
    identb = np.eye(128).astype(ml_dtypes.bfloat16)

    # globally-never-updated classes: the b-sharded bulk block (replicated)
    never = np.nonzero(counts == 0)[0]
    NBREAL = len(never)
    NBK = max(128, int(-(-NBREAL // 128)) * 128)
    prg = np.zeros((NBK, D), dtype=np.float32)
    prg[:NBREAL] = pr[never]
    prTg = np.ascontiguousarray(prg.T).astype(ml_dtypes.bfloat16)

    upds, n_us, chunk_members = [], [], []
    L = 16
    for j in range(N_CORES):
        members = np.nonzero(shard_of == j)[0]
        upd = np.unique(lab[members] - j * CSH)
        upds.append(upd)
        n_us.append(len(upd))
        per_chunk = [members[(members % BSH) // 128 == t] for t in range(LTB)]
        chunk_members.append(per_chunk)
        L = max(L, max(len(m) for m in per_chunk))
    # multiple of 32 so per-chunk idx slices start on even i16-pair columns
    # (odd 16-col offsets misalign the gpsimd gather ucode's index reads)
    L = int(-(-L // 32)) * 32
    NOWN = 4 * L
    NUPAD = max(128, int(-(-max(n_us) // 128)) * 128)

    in_maps, meta = [], []
    for j in range(N_CORES):
        upd, n_u = upds[j], n_us[j]
        inv = np.zeros(CSH, dtype=np.int64)
        inv[upd] = np.arange(n_u)

        # SEL: one-hot with EMA scale folded in; chunk-major dram layout
        SEL = np.zeros((B, NUPAD), dtype=np.float64)
        mine = np.nonzero(shard_of == j)[0]
        SEL[mine, inv[lab[mine] - j * CSH]] = sscale[lab[mine]]
        sel_dram = (SEL.reshape(N_CORES, LTB, 128, NUPAD)
                    .transpose(2, 1, 0, 3).reshape(128, NTB * NUPAD)
                    .astype(ml_dtypes.bfloat16))

        # old prototypes of the updated classes; pads -> e0 (finite norms)
        prj = np.zeros((NUPAD, D), dtype=np.float32)
        prj[:n_u] = pr[j * CSH + upd]
        prj[n_u:, 0] = 1.0
        prT = np.ascontiguousarray(prj.T).astype(ml_dtypes.bfloat16)

        # member gathers: per-chunk local columns of gTw; slots into ptn
        mg = np.zeros(NOWN, dtype=np.int64)
        ops = np.zeros(NOWN, dtype=np.int64)
        pos_b = np.full(NOWN, -1, dtype=np.int64)
        for t in range(LTB):
            m_t = chunk_members[j][t]
            n_t = len(m_t)
            mg[t * L:t * L + n_t] = (t * CHW + (m_t // BSH) * 128
                                      + (m_t % 128))
            ops[t * L:t * L + n_t] = inv[lab[m_t] - j * CSH]
            pos_b[t * L:t * L + n_t] = m_t

        in_maps.append({
            "gs": gs[BSH * j:BSH * (j + 1)].reshape(BSH * 4, QR),
            "sel": sel_dram,
            "prTb": prT,
            "prTg": prTg,
            "sv": svol[BSH * j:BSH * (j + 1)].reshape(128, BSH // 128),
            "mgi": _wrap16(mg),
            "opi": _wrap16(ops),
            "idn": ident,
            "idnb": identb,
            "osel": osel,
        })
        meta.append((pos_b, ops, n_u))

    return in_maps, meta, L, NUPAD, NBK, NBREAL


def _finish(results, meta, L, NUPAD):
    NCB = NUPAD // 128
    sum_d = 0.0
    sum_v = 0.0
    sum_v2 = 0.0
    d_own_all = np.empty(B, dtype=np.float64)
    n_total = 0
    for j in range(N_CORES):
        po = results[j]["po"].astype(np.float64)
        oo = results[j]["oo"].astype(np.float64).ravel()
        pos_b, ops, n_u = meta[j]
        sum_d += po[:, 0].sum()
        sum_v += po[:, 1].sum()
        sum_v2 += po[:, 2].sum()
        NFG = -(-GCOL // 1536)
        for cb in range(NCB):
            vr = min(128, max(0, n_u - cb * 128))
            sum_d += po[0:vr, 3 + cb * NFG:3 + (cb + 1) * NFG].sum()
        rinv2 = po[:, 16:16 + NCB].T.ravel()   # [slot] = 2/||ptil_slot||
        valid = pos_b >= 0
        bsel = pos_b[valid]
        gp = (-oo[valid]) * rinv2[ops[valid]] / 2.0 / GSCALE
        d_own_all[bsel] = np.sqrt(np.maximum(0.0, 2.0 - 2.0 * gp))
        n_total += valid.sum()
    assert n_total == B

    intra = d_own_all.mean()
    viol_all = 2.0 * B * C - sum_d
    viol_own = np.maximum(0.0, 2.0 - d_own_all).sum()
    inter = (viol_all - viol_own) / (B * (C - 1))
    mean_v = sum_v / B
    var_v = max((sum_v2 - B * mean_v * mean_v) / (B - 1), 0.0)
    vdl = -np.sqrt(var_v)
    cr = -mean_v
    total = 1.0 * intra + 2.0 * inter + 0.5 * vdl + 0.1 * cr
    return (np.float32(total), np.float32(intra), np.float32(inter),
            np.float32(vdl), np.float32(cr), np.float32(intra))


USE_F32R = True


def kernel(geometric_stream, simplex_volumes, prototypes, labels):
    in_maps, meta, L, NUPAD, NBK, NBREAL = _prep(
        geometric_stream, simplex_volumes, prototypes, labels)
    nc = _build(L, NUPAD, NBK, NBREAL, USE_F32R)
    res = run_bass_kernel_spmd(nc, in_maps, core_ids=list(range(N_CORES)))
    return _finish(res.results, meta, L, NUPAD)


# revision 29
# speedup vs baseline: 1.2856x; 1.2856x over previous
"""Trainium2 Bass kernel for nn_PrototypicalGeometricLoss (v3).

Strategy (8 NeuronCores, single NEFF launch):

  - Data-parallel streaming: each core mean-pools + L2-normalizes its
    B/8 = 512 rows of geometric_stream (bf16 in-flight DMA cast, PE pooling
    against a constant selection matrix) and computes distances to the
    never-updated classes (bulk) during the stream; phase A + bulk hide
    completely under the 41us DMA floor.
  - ONE post-stream AllGather of the normalized embeddings in fp8e4m3
    ([b,d] orientation, 512KB wire); collectives fired mid-stream contend
    with the saturated HBM path, and per-collective fixed cost dominates,
    so a single late gather beats four pipelined ones.  The assemble DMA
    (gpsimd SWDGE) casts fp8 -> bf16 in flight.
  - Segment sums for the EMA update are PE matmuls against a host-built
    one-hot selection matrix SEL [4096, NUPAD] (bf16) with the per-class
    EMA scale (1-m)/(m*count) folded into its values; old prototypes are
    injected into the same PSUM accumulation by an identity-stationary
    matmul, so ptil = p_old + scaled sums appears directly in PSUM (no
    gather/scan chain, no separate EMA add).  [d,b]-oriented tiles for the
    distance matmuls come from 32 PE transposes of the gathered tiles
    (dma_start_transpose races the collective - untracked dependency).
  - Renormalization avoids a broadcast matrix: phase F runs in swapped
    orientation (classes on partitions), so 2/||ptil_c|| is a per-partition
    activation `scale` vector (column-sum matmuls + Sqrt(0.25 x) + DVE
    reciprocal), and pad-class rows are excluded exactly by per-block host
    slicing.
  - Own-class dots: member columns of gTw (f32 copy of gathered gT)
    gathered once on gpsimd, multiplied elementwise with gathered -ptil
    columns, column-summed by a ones matmul; raw dots + per-class
    2/||ptil|| return to the host, which finishes in float64.
  - All post-collective work is emitted after the stream loop in
    dependency-monotone order: engine queues are in-order, and ~2.5us
    cross-engine latency per dependent instruction makes a short, batched
    tail the dominant design constraint.
"""

import functools
import sys

sys.path.insert(0, "/opt/trn_rl_repo")

import numpy as np
import ml_dtypes

import concourse.bass as bass  # noqa: F401
import concourse.bacc as bacc
import concourse.mybir as mybir
from concourse import tile
from concourse.bass_utils import run_bass_kernel_spmd

N_CORES = 8
B, S, D, C = 4096, 64, 128, 10000
BSH = B // N_CORES           # 512 batch rows per core
LTB = BSH // 128             # 4 local b-tiles
NTB = B // 128               # 32 global b-tiles
CSH = C // N_CORES           # 1250 classes per core
GCOL = NTB * 128             # 4096 columns of gT
MOM = 0.9
GSCALE = 1.0 - 1e-6          # keeps 2 - 2*g.p strictly positive
QR = 2048                    # gs quarter-row length (16 s x 128 d)
CHW = 1024                   # gT columns per AllGather chunk

f32 = mybir.dt.float32
fp8 = mybir.dt.float8e4
bf16 = mybir.dt.bfloat16
i16 = mybir.dt.int16
AF = mybir.ActivationFunctionType
ALU = mybir.AluOpType
AX = mybir.AxisListType


def _wrap16(flat):
    """Lay a flat index list out in the GPSIMD wrapped-by-16 format."""
    n = flat.shape[0]
    assert n % 16 == 0
    w = flat.reshape(n // 16, 16).T.astype(np.int16)   # [16, n//16]
    return np.tile(w, (8, 1))                          # [128, n//16]


@functools.lru_cache(maxsize=16)
def _build(L, NUPAD, NBK, NBREAL, use_f32r=True, upto=99, unroll=1,
           skip_ag=False, dma_only=False, dma2x=False):
    """Build + compile the SPMD program.

    L = padded member count per AllGather chunk; NUPAD = padded
    updated-classes-per-shard block; NBK/NBREAL = padded/real count of
    globally-never-updated classes (the b-sharded bulk pass)."""
    NOWN = 4 * L
    NCB = NUPAD // 128                    # class blocks (partition tiles)
    # po packs per-(class-block, group) F sums at cols 3..3+NCB*NFG and
    # rinv2 at 16..16+NCB; bigger NUPAD must fail loudly here
    assert NCB <= 4, "po column layout requires NCB <= 4"
    bulk_chunks = []                      # (c0, width) into prTg
    for c0 in range(0, NBK, 1536):
        bulk_chunks.append((c0, min(1536, NBK - c0)))
    NBG = LTB * len(bulk_chunks)
    f_groups = []                         # (k0, width) into gT columns
    for k0 in range(0, GCOL, 1536):
        f_groups.append((k0, min(1536, GCOL - k0)))
    NFG = len(f_groups)                   # 3 groups of <=1536 cols
    NAC = NBG

    nc = bacc.Bacc("TRN2", target_bir_lowering=False, debug=False,
                   num_devices=N_CORES)

    gs = nc.dram_tensor("gs", [BSH * 4, QR], f32, kind="ExternalInput")
    sel = nc.dram_tensor("sel", [128, NTB * NUPAD], bf16,
                         kind="ExternalInput")
    prTb = nc.dram_tensor("prTb", [128, NUPAD], bf16, kind="ExternalInput")
    prTg = nc.dram_tensor("prTg", [128, NBK], bf16, kind="ExternalInput")
    sv = nc.dram_tensor("sv", [128, BSH // 128], f32, kind="ExternalInput")
    mgi = nc.dram_tensor("mgi", [128, NOWN // 16], i16, kind="ExternalInput")
    opi = nc.dram_tensor("opi", [128, NOWN // 16], i16, kind="ExternalInput")
    idn = nc.dram_tensor("idn", [128, 128], f32, kind="ExternalInput")
    idnb = nc.dram_tensor("idnb", [128, 128], bf16, kind="ExternalInput")
    osel = nc.dram_tensor("osel", [128, 32], bf16, kind="ExternalInput")
    po = nc.dram_tensor("po", [128, 20], f32, kind="ExternalOutput")
    oo = nc.dram_tensor("oo", [1, NOWN], f32, kind="ExternalOutput")

    with tile.TileContext(nc) as tc:
        with (
            tc.tile_pool(name="dram", bufs=1, space="DRAM") as dram,
            tc.tile_pool(name="consts", bufs=1) as consts,
            tc.tile_pool(name="gbig", bufs=1) as gbig,
            tc.tile_pool(name="slab", bufs=8) as slabp,
            tc.tile_pool(name="slabf", bufs=2) as slabfp,
            tc.tile_pool(name="selp", bufs=2) as selp,
            tc.tile_pool(name="norm", bufs=2) as normp,
            tc.tile_pool(name="glocp", bufs=4) as glocp,
            tc.tile_pool(name="ps_small", bufs=1, space="PSUM") as ps_small,
            tc.tile_pool(name="ps_big", bufs=2, space="PSUM") as ps_big,
            tc.tile_pool(name="ps_seg", bufs=1, space="PSUM") as ps_seg,
            tc.tile_pool(name="dscr", bufs=2) as dscrp,
            tc.tile_pool(name="outs", bufs=1) as outsp,
        ):
            for it in range(unroll):
                if it > 0:
                    tc.strict_bb_all_engine_barrier()
                ag_in = dram.tile([128, LTB * 128], fp8, name="ag_in",
                                  tag="ag_in")
                ag_out = dram.tile([N_CORES, 128, LTB * 128], fp8,
                                   name="ag_out", addr_space="Shared",
                                   tag="ag_out")
                ident = consts.tile([128, 128], f32, name="ident")
                nc.sync.dma_start(ident[:, :], idn[:, :])
                identb = consts.tile([128, 128], bf16, name="identb")
                nc.sync.dma_start(identb[:, :], idnb[:, :])
                osel_sb = consts.tile([128, 32], bf16, name="osel_sb")
                nc.sync.dma_start(osel_sb[:, :], osel[:, :])
                mgi_sb = consts.tile([128, NOWN // 16], i16, name="mgi_sb")
                nc.sync.dma_start(mgi_sb[:, :], mgi[:, :])
                opi_sb = consts.tile([128, NOWN // 16], i16, name="opi_sb")
                nc.sync.dma_start(opi_sb[:, :], opi[:, :])
                sv_sb = consts.tile([128, BSH // 128], f32, name="sv_sb")
                nc.sync.dma_start(sv_sb[:, :], sv[:, :])
                bias2 = consts.tile([128, 1], f32, name="bias2")
                nc.vector.memset(bias2[:, :], 2.0)
                onescol = consts.tile([128, 1], f32, name="onescol")
                nc.vector.memset(onescol[:, :], 1.0)
                prT_sb = consts.tile([128, NUPAD], bf16, name="prT_sb")
                nc.sync.dma_start(prT_sb[:, :], prTb[:, :])
                prTg_sb = consts.tile([128, NBK], bf16, name="prTg_sb")
                nc.scalar.dma_start(prTg_sb[:, :], prTg[:, :])

                gT = gbig.tile([128, GCOL], bf16, name="gT")
                gTw = gbig.tile([128, GCOL], f32, name="gTw")
                gnAll = gbig.tile([128, GCOL], bf16, name="gnAll")
                mems = gbig.tile([128, NOWN], f32, name="mems")
                acc = outsp.tile([128, NAC], f32, name="acc")
                po_sb = outsp.tile([128, 20], f32, name="po_sb")
                oo_sb = outsp.tile([1, NOWN], f32, name="oo_sb")
                if upto < 1:
                    nc.vector.memset(acc[:, 0:NBG], 0.0)

                seg_ps = ps_seg.tile([128, NUPAD], f32, name="seg_ps")
                if upto >= 3:
                    # inject p_old into the EMA accumulation group
                    nc.tensor.matmul(seg_ps[:, :], identb[:, :], prT_sb[:, :],
                                     start=True, stop=False,
                                     skip_group_check=True)


                def transp(t):
                    for j in range(N_CORES):
                        blk = j * LTB + t
                        pst2 = ps_big.tile([128, 128], bf16, name="pst2",
                                           tag="psb")
                        nc.tensor.transpose(pst2[:, :],
                                            gnAll[:, blk * 128:
                                                  (blk + 1) * 128],
                                            identb[:, :])
                        nc.vector.tensor_scalar_mul(
                            gT[:, blk * 128:(blk + 1) * 128], pst2[:, :], 1.0)

                def memgather():
                    nc.gpsimd.ap_gather(mems[:, :], gTw[:, :],
                                        mgi_sb[:, :],
                                        channels=128, num_elems=GCOL, d=1,
                                        num_idxs=NOWN)

                def segsum(t, sel_sb):
                    for blk in range(N_CORES):
                        g0 = blk * (LTB * 128) + t * 128
                        nc.tensor.matmul(
                            seg_ps[:, :],
                            gnAll[:, g0:g0 + 128],
                            sel_sb[:, blk * NUPAD:(blk + 1) * NUPAD],
                            start=False,
                            stop=(t == LTB - 1 and blk == N_CORES - 1),
                            skip_group_check=True)

                # ---- Phase A: stream + PE-pool + normalize + AG + bulk
                sel_tiles = []
                glocs = []
                for t in range(LTB if (upto >= 1 or dma_only) else 0):
                    if upto >= 3:
                        sel_sb = selp.tile([128, N_CORES * NUPAD], bf16,
                                           name="sel_sb", tag="sel")
                        nc.sync.dma_start(
                            sel_sb[:, :],
                            sel[:, t * N_CORES * NUPAD:
                                (t + 1) * N_CORES * NUPAD])
                        sel_tiles.append(sel_sb)
                    pooled = ps_small.tile([128, 128], f32, name="pooled",
                                           tag="pst")
                    hs = []
                    for h in range(2):
                        sh = slabp.tile([128, 2 * QR], bf16, name="sh",
                                        tag="slab")
                        r0 = t * 512 + h * 256
                        nc.gpsimd.dma_start(
                            sh[:, :].rearrange("p (tl f) -> p tl f", tl=2),
                            gs[r0:r0 + 256, :].rearrange(
                                "(tl p) f -> p tl f", tl=2))
                        if dma2x:
                            # probe: duplicate read on the SP HWDGE queue
                            # (f32, no cast) to test dual-queue bandwidth
                            sf = slabfp.tile([128, 2 * QR], f32, name="sf",
                                             tag="slabf")
                            nc.sync.dma_start(
                                sf[:, :].rearrange("p (tl f) -> p tl f",
                                                   tl=2),
                                gs[r0:r0 + 256, :].rearrange(
                                    "(tl p) f -> p tl f", tl=2))
                        hs.append(sh)
                    if dma_only:
                        continue
                    for tl in range(4):
                        sh = hs[tl // 2]
                        o = (tl % 2) * QR
                        for q in range(16):
                            nc.tensor.matmul(
                                pooled[tl * 32:(tl + 1) * 32, :],
                                osel_sb[:, :],
                                sh[:, o + q * 128:o + (q + 1) * 128],
                                start=(q == 0), stop=(q == 15),
                                tile_position=(0, tl * 32))
                    scr = normp.tile([128, 128], f32, name="scr", tag="scr")
                    ssq = normp.tile([128, 1], f32, name="ssq", tag="ssq")
                    nc.scalar.activation(scr[:, :], pooled[:, :], AF.Square,
                                         accum_out=ssq[:, :])
                    nrm = normp.tile([128, 1], f32, name="nrm", tag="nrm")
                    nc.scalar.activation(nrm[:, :], ssq[:, :], AF.Sqrt)
                    rcp = normp.tile([128, 1], f32, name="rcp", tag="rcp")
                    nc.vector.reciprocal(rcp[:, :], nrm[:, :])
                    gnb = normp.tile([128, 128], fp8, name="gnb", tag="gnb")
                    nc.vector.tensor_scalar(gnb[:, :], pooled[:, :],
                                            rcp[:, :], GSCALE, ALU.mult,
                                            ALU.mult)
                    gn = normp.tile([128, 128], f32, name="gn", tag="gn")
                    nc.vector.tensor_scalar(gn[:, :], pooled[:, :], rcp[:, :],
                                            GSCALE, ALU.mult, ALU.mult)
                    pst = ps_small.tile([128, 128], f32, name="pst", tag="pst")
                    nc.tensor.transpose(pst[:, :], gn[:, :], ident[:, :])
                    gloc = glocp.tile([128, 128], bf16, name="gloc",
                                      tag="gloc")
                    nc.scalar.activation(gloc[:, :], pst[:, :], AF.Copy)
                    nc.sync.dma_start(ag_in[:, t * 128:(t + 1) * 128],
                                      gnb[:, :])
                    glocs.append(gloc)

                    # bulk: local b-tile x never-updated classes
                    for ci, (c0, cw) in enumerate(bulk_chunks):
                        psb = ps_big.tile([128, 1536], f32, name="psb",
                                          tag="psb")
                        for d0 in range(0, cw, 512):
                            dn = min(512, cw - d0)
                            nc.tensor.matmul(
                                psb[:, d0:d0 + dn],
                                gloc[:, :],
                                prTg_sb[:, c0 + d0:c0 + d0 + dn],
                                start=True, stop=True)
                        rw = min(cw, max(0, NBREAL - c0))
                        dscb = dscrp.tile([128, 1536], bf16, name="dscb",
                                          tag="dsc")
                        gidx = t * len(bulk_chunks) + ci
                        nc.scalar.activation(dscb[:, 0:rw], psb[:, 0:rw],
                                             AF.Sqrt, bias=bias2[:, :],
                                             scale=-2.0,
                                             accum_out=acc[:, gidx:gidx + 1])

                # post-stream: assemble chunks, then chunk segsums in
                # dependency order (in-order engine queues must not stall
                # behind collective waits during the stream; deps are
                # monotone in t, so queues drain without head blocking)
                if upto >= 2 and not dma_only:
                    # one collective fired post-stream: pays the per-AG fixed
                    # cost once; everything downstream waits on the last tile
                    # anyway and is latency-bound
                    if not skip_ag:
                        nc.gpsimd.collective_compute(
                            "AllGather", ALU.bypass,
                            replica_groups=[list(range(N_CORES))],
                            ins=[ag_in.opt()], outs=[ag_out.opt()])
                    nc.gpsimd.dma_start(
                        gnAll[:, :].rearrange("p (j w) -> p j w",
                                              j=N_CORES),
                        ag_out[:, :, :].rearrange("j p w -> p j w"))
                    if upto >= 3:
                        for t in range(LTB):
                            segsum(t, sel_tiles[t])

                # bulk reduce: input complete at stream end; emitted
                # here so it executes during the AllGather wait
                nc.vector.tensor_reduce(po_sb[:, 0:1], acc[:, 0:NBG], AX.X,
                                        ALU.add)

                # ---- ptil -> norms -> rinv2 (per-class 2/||ptil||)
                ptilb = gbig.tile([128, NUPAD], bf16, name="ptilb")
                ptn = gbig.tile([128, NUPAD], f32, name="ptn")
                sqt = gbig.tile([128, NUPAD], f32, name="sqt")
                srow = outsp.tile([128, NCB], f32, name="srow")
                if upto >= 3:
                    nc.vector.tensor_scalar_mul(ptilb[:, :], seg_ps[:, :],
                                                -1.0)
                    nc.vector.tensor_scalar_mul(ptn[:, :], seg_ps[:, :], -1.0)
                    nc.scalar.activation(sqt[:, :], ptilb[:, :], AF.Square)
                    nps = ps_small.tile([128, NCB], f32, name="nps",
                                        tag="pst")
                    for cb in range(NCB):
                        nc.tensor.matmul(nps[:, cb:cb + 1],
                                         sqt[:, cb * 128:(cb + 1) * 128],
                                         onescol[:, :],
                                         start=True, stop=True)
                    nc.scalar.activation(srow[:, :], nps[:, :], AF.Sqrt,
                                         scale=0.25)
                    nc.vector.reciprocal(po_sb[:, 16:16 + NCB], srow[:, :])
                else:
                    nc.vector.memset(ptilb[:, :], 0.0)
                    nc.vector.memset(ptn[:, :], 0.0)
                    nc.vector.memset(po_sb[:, 16:16 + NCB], 0.0)

                if upto >= 2 and not dma_only:
                    for t in range(LTB):
                        transp(t)
                    nc.vector.tensor_scalar_mul(gTw[:, :], gT[:, :], 1.0)
                    memgather()

                # ---- simplex volume partials
                nc.vector.tensor_reduce(po_sb[:, 1:2], sv_sb[:, :], AX.X,
                                        ALU.add)
                junk1 = outsp.tile([128, BSH // 128], f32, name="junk1")
                nc.vector.scalar_tensor_tensor(junk1[:, :], sv_sb[:, :], 1.0,
                                               sv_sb[:, :], ALU.mult,
                                               ALU.mult,
                                               accum_out=po_sb[:, 2:3])
                nc.vector.memset(po_sb[:, 3 + NCB * NFG:16], 0.0)

                # ---- Phase F: distances for updated classes (swapped)
                if upto < 4:
                    nc.vector.memset(po_sb[:, 3:3 + NCB * NFG], 0.0)
                for cb in range(NCB if upto >= 4 else 0):
                    for gi, (k0, w) in enumerate(f_groups):
                        psF = ps_big.tile([128, 1536], f32, name="psF",
                                          tag="psb")
                        for d0 in range(0, w, 512):
                            dn = min(512, w - d0)
                            nc.tensor.matmul(
                                psF[:, d0:d0 + dn],
                                ptilb[:, cb * 128:(cb + 1) * 128],
                                gT[:, k0 + d0:k0 + d0 + dn],
                                start=True, stop=True)
                        dscf = dscrp.tile([128, 1536], bf16, name="dscf",
                                          tag="dsc")
                        aidx = 3 + cb * NFG + gi
                        nc.scalar.activation(dscf[:, 0:w], psF[:, 0:w],
                                             AF.Sqrt, bias=bias2[:, :],
                                             scale=po_sb[:, 16 + cb:17 + cb],
                                             accum_out=po_sb[:, aidx:aidx + 1])

                # ---- own-class raw dots: mems . gathered(-ptil)
                opg = gbig.tile([128, NOWN], f32, name="opg")
                prod = gbig.tile([128, NOWN], f32, name="prod")
                if upto >= 5:
                    nc.gpsimd.ap_gather(opg[:, :], ptn[:, :], opi_sb[:, :],
                                        channels=128, num_elems=NUPAD, d=1,
                                        num_idxs=NOWN)
                    nc.vector.tensor_tensor(prod[:, :], mems[:, :],
                                            opg[:, :], ALU.mult)
                    for o0 in range(0, NOWN, 512):
                        on = min(512, NOWN - o0)
                        dps = ps_big.tile([1, 512], f32, name="dps",
                                          tag="psb")
                        nc.tensor.matmul(dps[0:1, 0:on], onescol[:, :],
                                         prod[:, o0:o0 + on],
                                         start=True, stop=True)
                        nc.vector.tensor_scalar_mul(oo_sb[0:1, o0:o0 + on],
                                                    dps[0:1, 0:on], 1.0)
                else:
                    nc.vector.memset(oo_sb[:, :], 0.0)
                nc.sync.dma_start(oo[:, :], oo_sb[:, :])

                nc.sync.dma_start(po[:, :], po_sb[:, :])

    nc.compile()
    return nc


def _prep(geometric_stream, simplex_volumes, prototypes, labels):
    gs = np.ascontiguousarray(np.asarray(geometric_stream, dtype=np.float32))
    svol = np.ascontiguousarray(np.asarray(simplex_volumes, dtype=np.float32))
    pr = np.asarray(prototypes, dtype=np.float32)
    lab = np.asarray(labels).astype(np.int64).ravel()
    assert gs.shape == (B, S, D) and pr.shape == (C, D) and lab.shape == (B,)

    counts = np.bincount(lab, minlength=C)
    sscale = ((1.0 - MOM) / np.maximum(counts, 1.0) / MOM).astype(np.float64)

    shard_of = lab // CSH
    # osel[p, m] = 1 iff p//4 == m  (sums the 4 s-quarters of batch m)
    osel = (np.arange(128)[:, None] // 4 == np.arange(32)[None, :])
    osel = osel.astype(ml_dtypes.bfloat16)
    ident = np.eye(128, dtype=np.float32)
    identb = np.eye(128).astype(ml_dtypes.bfloat16)

    # globally-never-updated classes: the b-sharded bulk block (replicated)
    never = np.nonzero(counts == 0)[0]
    NBREAL = len(never)
    NBK = max(128, int(-(-NBREAL // 128)) * 128)
    prg = np.zeros((NBK, D), dtype=np.float32)
    prg[:NBREAL] = pr[never]
    prTg = np.ascontiguousarray(prg.T).astype(ml_dtypes.bfloat16)

    upds, n_us, chunk_members = [], [], []
    L = 16
    for j in range(N_CORES):
        members = np.nonzero(shard_of == j)[0]
        upd = np.unique(lab[members] - j * CSH)
        upds.append(upd)
        n_us.append(len(upd))
        per_chunk = [members[(members % BSH) // 128 == t] for t in range(LTB)]
        chunk_members.append(per_chunk)
        L = max(L, max(len(m) for m in per_chunk))
    # multiple of 32 so per-chunk idx slices start on even i16-pair columns
    # (odd 16-col offsets misalign the gpsimd gather ucode's index reads)
    L = int(-(-L // 32)) * 32
    NOWN = 4 * L
    NUPAD = max(128, int(-(-max(n_us) // 128)) * 128)

    in_maps, meta = [], []
    for j in range(N_CORES):
        upd, n_u = upds[j], n_us[j]
        inv = np.zeros(CSH, dtype=np.int64)
        inv[upd] = np.arange(n_u)

        # SEL: one-hot with EMA scale folded in; chunk-major dram layout
        SEL = np.zeros((B, NUPAD), dtype=np.float64)
        mine = np.nonzero(shard_of == j)[0]
        SEL[mine, inv[lab[mine] - j * CSH]] = sscale[lab[mine]]
        sel_dram = (SEL.reshape(N_CORES, LTB, 128, NUPAD)
                    .transpose(2, 1, 0, 3).reshape(128, NTB * NUPAD)
                    .astype(ml_dtypes.bfloat16))

        # old prototypes of the updated classes; pads -> e0 (finite norms)
        prj = np.zeros((NUPAD, D), dtype=np.float32)
        prj[:n_u] = pr[j * CSH + upd]
        prj[n_u:, 0] = 1.0
        prT = np.ascontiguousarray(prj.T).astype(ml_dtypes.bfloat16)

        # member gathers: per-chunk local columns of gTw; slots into ptn
        mg = np.zeros(NOWN, dtype=np.int64)
        ops = np.zeros(NOWN, dtype=np.int64)
        pos_b = np.full(NOWN, -1, dtype=np.int64)
        for t in range(LTB):
            m_t = chunk_members[j][t]
            n_t = len(m_t)
            mg[t * L:t * L + n_t] = m_t
            ops[t * L:t * L + n_t] = inv[lab[m_t] - j * CSH]
            pos_b[t * L:t * L + n_t] = m_t

        in_maps.append({
            "gs": gs[BSH * j:BSH * (j + 1)].reshape(BSH * 4, QR),
            "sel": sel_dram,
            "prTb": prT,
            "prTg": prTg,
            "sv": svol[BSH * j:BSH * (j + 1)].reshape(128, BSH // 128),
            "mgi": _wrap16(mg),
            "opi": _wrap16(ops),
            "idn": ident,
            "idnb": identb,
            "osel": osel,
        })
        meta.append((pos_b, ops, n_u))

    return in_maps, meta, L, NUPAD, NBK, NBREAL


def _finish(results, meta, L, NUPAD):
    NCB = NUPAD // 128
    sum_d = 0.0
    sum_v = 0.0
    sum_v2 = 0.0
    d_own_all = np.empty(B, dtype=np.float64)
    n_total = 0
    for j in range(N_CORES):
        po = results[j]["po"].astype(np.float64)
        oo = results[j]["oo"].astype(np.float64).ravel()
        pos_b, ops, n_u = meta[j]
        sum_d += po[:, 0].sum()
        sum_v += po[:, 1].sum()
        sum_v2 += po[:, 2].sum()
        NFG = -(-GCOL // 1536)
        for cb in range(NCB):
            vr = min(128, max(0, n_u - cb * 128))
            sum_d += po[0:vr, 3 + cb * NFG:3 + (cb + 1) * NFG].sum()
        rinv2 = po[:, 16:16 + NCB].T.ravel()   # [slot] = 2/||ptil_slot||
        valid = pos_b >= 0
        bsel = pos_b[valid]
        gp = (-oo[valid]) * rinv2[ops[valid]] / 2.0 / GSCALE
        d_own_all[bsel] = np.sqrt(np.maximum(0.0, 2.0 - 2.0 * gp))
        n_total += valid.sum()
    assert n_total == B

    intra = d_own_all.mean()
    viol_all = 2.0 * B * C - sum_d
    viol_own = np.maximum(0.0, 2.0 - d_own_all).sum()
    inter = (viol_all - viol_own) / (B * (C - 1))
    mean_v = sum_v / B
    var_v = max((sum_v2 - B * mean_v * mean_v) / (B - 1), 0.0)
    vdl = -np.sqrt(var_v)
    cr = -mean_v
    total = 1.0 * intra + 2.0 * inter + 0.5 * vdl + 0.1 * cr
    return (np.float32(total), np.float32(intra), np.float32(inter),
            np.float32(vdl), np.float32(cr), np.float32(intra))


USE_F32R = True


def kernel(geometric_stream, simplex_volumes, prototypes, labels):
    in_maps, meta, L, NUPAD, NBK, NBREAL = _prep(
        geometric_stream, simplex_volumes, prototypes, labels)
    nc = _build(L, NUPAD, NBK, NBREAL, USE_F32R)
    res = run_bass_kernel_spmd(nc, in_maps, core_ids=list(range(N_CORES)))
    return _finish(res.results, meta, L, NUPAD)
